# revision 21
# baseline (speedup 1.0000x reference)
"""Trainium2 Bass kernel for nn_CycleNet_EPD (ragged graph edge-phase decoder).

Math (per graph b, La = edge_len[b], Ba = beta_len[b]):
  ef[e,:4]   = [x[src_e], x[dst_e]]
  s[beta,:]  = sum_e |SCB[b,beta,e]| * ef[e,:]
  emb        = relu(s@W1+b1)@W2+b2
  A[beta,:]  = emb@W3a + b3              (W3a=W3[:64])
  z_b[:,e]   = A[:,b] + |SCB[b,b,e]| * (W3b^T ef[e])
  H[e,:]     = sum_b relu(z_b)
  out[e,:]   = relu((H@W4 + vb)@W5+b5)@W6+b6
               vb = 64*b4 + (64-Ba)*relu(A_pad)@W4
  rows with e >= La are zero.

v2 design — fp8 DoubleRow hot loop, bias folded into the matmul:
  - Host precomputes the tiny per-graph prolog (s, emb, A, vb) and packs
    esp[k-slot, kt, e] = fp8(|scb|*ef) with a ones-row; the per-beta
    stationary weights are masked [32,2,128] fp8 blocks holding W3b rows
    plus the A-bias split hi+lo across the two ones-slots.  Each beta
    occupies 3 rows of one 32-partition quadrant (10 betas/quadrant,
    30/layer over quadrants 0/32/64), so one DoubleRow matmul computes
    z_b = A_b + |scb|*G for 128 channels directly in PSUM.
  - Betas are processed in pairs into a 2-bank PSUM pair tile; a single
    relu op per pair (alternating ACT activation / DVE tensor_scalar max,
    ratio ALPHA) writes fp8 r-pairs [128,2,n] in SBUF.
  - One fp8 DoubleRow matmul per pair with stacked (W4,W4) weights
    accumulates sum_b relu(z_b) @ W4 into PSUM (beta-sum on the PE,
    no vector adds at all).
  - Out stage per 512-e chunk: r5 = pW4+vb (DVE), W5 (bf16 mm),
    r6 = relu(+b5) (ACT), W6 (bf16 mm), out = +b6 (ACT, fp32), DMA out
    in [h,e] layout; the host transposes rows during the unshard.

Sharding: per-core work items (graph, e0, e1) balanced by
(Ba+OUT_W)*ne + GFIX; one NEFF, per-core schedule in a partition-id
switch.  Host does gathers/abs/products/packing/casts + the tiny MLP
prolog; all O(Ba*La*128) math runs on the device.
"""

import os
import sys

sys.path.insert(0, "/opt/trn_rl_repo")

import ml_dtypes
import numpy as np

import concourse.bacc as bacc
import concourse.mybir as mybir
import concourse.tile as tile
from concourse import bass_utils

B, MAX_N, MAX_E, MAX_BETA = 16, 512, 1024, 64
NODE_F, HID = 2, 128
NCORES = 8
F32 = mybir.dt.float32
BF16 = mybir.dt.bfloat16
F8 = mybir.dt.float8e4
AF = mybir.ActivationFunctionType
ALU = mybir.AluOpType
PM = mybir.MatmulPerfMode
NPBF16 = ml_dtypes.bfloat16
NP8 = ml_dtypes.float8_e4m3

ECHUNK = 512
OUT_W = 8      # planner: out-stage cost per edge column (beta-col units)
GFIX = 2000    # planner: per-slice fixed cost
ALPHA = 0.54   # fraction of relu pairs on ACT (rest on DVE)


def _q8(a):
    return np.clip(a, -240, 240).astype(NP8)


def _plan(edge_len, beta_len):
    """Per-core work items (g, e0, e1); large graphs split by edge range."""
    La = [max(1, min(MAX_E, int(v))) for v in edge_len]
    Ba = [max(1, min(MAX_BETA, int(v))) for v in beta_len]

    def el(g, ne):
        return (Ba[g] + OUT_W) * ne + GFIX

    total = sum(el(g, La[g]) for g in range(B))
    target = total / NCORES
    pieces = []
    for g in range(B):
        k = max(1, min(round(el(g, La[g]) / target + 0.25), -(-La[g] // 64)))
        base, rem = divmod(La[g], k)
        e0 = 0
        for j in range(k):
            ne = base + (1 if j < rem else 0)
            pieces.append((g, e0, e0 + ne))
            e0 += ne
    pieces.sort(key=lambda p: -el(p[0], p[2] - p[1]))
    cores = [[] for _ in range(NCORES)]
    loads = [0.0] * NCORES
    for p in pieces:
        c = min(range(NCORES), key=lambda i: loads[i])
        cores[c].append(p)
        loads[c] += el(p[0], p[2] - p[1])
    for _ in range(64):
        cM = max(range(NCORES), key=lambda i: loads[i])
        cm = min(range(NCORES), key=lambda i: loads[i])
        surplus = loads[cM] - loads[cm]
        best = None
        for idx, (g, e0, e1) in enumerate(cores[cM]):
            ne_mv = int((surplus / 2 - GFIX) / (Ba[g] + OUT_W))
            ne_mv = min(ne_mv, e1 - e0 - 64)
            if ne_mv >= 64 and (best is None or ne_mv > best[1]):
                best = (idx, ne_mv)
        if best is None:
            break
        idx, ne_mv = best
        g, e0, e1 = cores[cM][idx]
        cores[cM][idx] = (g, e0, e1 - ne_mv)
        cores[cm].append((g, e1 - ne_mv, e1))
        loads[cM] -= (Ba[g] + OUT_W) * ne_mv
        loads[cm] += el(g, ne_mv)
    return La, Ba, cores


def kernel(x, SCB, edge_index, edge_len, beta_len,
           W1, b1, W2, b2, W3, b3, W4, b4, W5, b5, W6, b6):
    x = np.asarray(x, np.float32)
    SCB = np.asarray(SCB, np.float32)
    edge_index = np.asarray(edge_index, np.int32)
    W1, b1 = np.asarray(W1, np.float32), np.asarray(b1, np.float32)
    W2, b2 = np.asarray(W2, np.float32), np.asarray(b2, np.float32)
    W3, b3 = np.asarray(W3, np.float32), np.asarray(b3, np.float32)
    W4, b4 = np.asarray(W4, np.float32), np.asarray(b4, np.float32)
    W5, b5 = np.asarray(W5, np.float32), np.asarray(b5, np.float32)
    W6, b6 = np.asarray(W6, np.float32), np.asarray(b6, np.float32)
    La, Ba, cores = _plan(np.asarray(edge_len), np.asarray(beta_len))
    W3a, W3b = W3[:64], W3[64:]
    W3b8 = _q8(W3b).astype(np.float32)

    # ---- host prolog per graph: ef, s, emb, A(hi/lo), vb ----
    ef_all, Ahi_all, Alo_all, vb_all, nlay_all = [], [], [], [], []
    K0 = np.maximum((np.maximum(b1, 0) @ W2 + b2) @ W3a + b3, 0.0)  # A_pad relu
    for g in range(B):
        la, ba = La[g], Ba[g]
        src, dst = edge_index[g, 0, :la], edge_index[g, 1, :la]
        ef = np.concatenate([x[g][src], x[g][dst]], axis=1)  # [la,4]
        ef_all.append(ef)
        asc = np.abs(SCB[g][:ba, :la])
        s = asc @ ef                                          # [ba,4]
        emb = np.maximum(s @ W1 + b1, 0.0) @ W2 + b2
        A = emb @ W3a + b3                                    # [ba,128]
        Ahi = _q8(A)
        Alo = _q8(A - Ahi.astype(np.float32))
        Ahi_all.append(Ahi)
        Alo_all.append(Alo)
        vb_all.append((64.0 - ba) * (K0 @ W4) + 64.0 * b4)    # [128]
        nlay_all.append(-(-ba // 30))

    # ---- per-core layouts ----
    # esp: [128, 2, CMAX] fp8, per slice cols [eoff + l*ne + e]
    # wblk: [128, 2, 128*SMAX] fp8, per slice slot base sbase
    sched = []   # per core: list of dicts per slice
    cmax = smax = omax = nslmax = 1
    for c in range(NCORES):
        eoff = soff = ooff = 0
        items = []
        for (g, e0, e1) in cores[c]:
            ne = e1 - e0
            nep = -(-ne // 8) * 8
            L = nlay_all[g]
            items.append(dict(g=g, e0=e0, e1=e1, ne=ne, nep=nep, L=L,
                              eoff=eoff, sbase=soff, ooff=ooff))
            eoff += L * nep
            soff += sum(-(-min(Ba[g] - 30 * l, 30) // 3) for l in range(L))
            ooff += nep
        sched.append(items)
        cmax = max(cmax, -(-eoff // 8) * 8)
        smax = max(smax, soff)
        omax = max(omax, ooff)
        nslmax = max(nslmax, len(items))

    in_maps = []
    for c in range(NCORES):
        esp = np.zeros((128, 2, cmax), NP8)
        wblk = np.zeros((128, smax, 2, 128), NP8)
        fcst = np.zeros((128, 2 + nslmax), np.float32)
        fcst[:, 0] = b5
        fcst[:, 1] = b6
        for si, it in enumerate(sched[c]):
            g, e0, e1, ne, L = it["g"], it["e0"], it["e1"], it["ne"], it["L"]
            fcst[:, 2 + si] = vb_all[g]
            ef = ef_all[g][e0:e1]                     # [ne,4]
            asc = np.abs(SCB[g][:Ba[g], e0:e1])       # [ba,ne]
            Ahi, Alo = Ahi_all[g], Alo_all[g]
            sb = it["sbase"]
            lbase = 0
            for l in range(L):
                nbl = min(Ba[g] - 30 * l, 30)
                for rr in range(nbl):
                    b = 30 * l + rr
                    q, j = rr % 3, rr // 3
                    col = it["eoff"] + l * it["nep"]
                    p0 = 32 * q + 3 * j
                    prod = _q8(asc[b][None, :] * ef.T)    # [4,ne]
                    esp[p0 + 0, 0, col:col + ne] = prod[0]
                    esp[p0 + 0, 1, col:col + ne] = prod[1]
                    esp[p0 + 1, 0, col:col + ne] = prod[2]
                    esp[p0 + 1, 1, col:col + ne] = prod[3]
                    esp[p0 + 2, 0, col:col + ne] = 1.0
                    esp[p0 + 2, 1, col:col + ne] = 1.0
                    sc = sb + lbase + j
                    wblk[p0 + 0, sc, 0] = W3b8[0]
                    wblk[p0 + 0, sc, 1] = W3b8[1]
                    wblk[p0 + 1, sc, 0] = W3b8[2]
                    wblk[p0 + 1, sc, 1] = W3b8[3]
                    wblk[p0 + 2, sc, 0] = Ahi[b]
                    wblk[p0 + 2, sc, 1] = Alo[b]
                lbase += -(-nbl // 3)
            it["lslots"] = [0] * L
            lb = 0
            for l in range(L):
                it["lslots"][l] = lb
                lb += -(-min(Ba[g] - 30 * l, 30) // 3)
        w4hi = _q8(W4)
        w4lo = _q8(W4 - w4hi.astype(np.float32))
        w4dr = np.zeros((128, 2, 2, 128), NP8)
        w4dr[:, 0, 0] = w4hi
        w4dr[:, 0, 1] = w4hi
        w4dr[:, 1, 0] = w4lo
        w4dr[:, 1, 1] = w4lo
        wcst = np.zeros((128, 256), np.float32)
        wcst[:, 0:128] = W5
        wcst[:, 128:256] = W6
        in_maps.append({
            "esp": esp, "wblk": wblk, "w4dr": w4dr,
            "wcst": wcst.astype(NPBF16), "fcst": fcst,
        })

    # ---- build program ----
    one_core = os.environ.get("KERNEL_ONE_CORE")
    ndev = 1 if one_core is not None else NCORES
    nc = bacc.Bacc("TRN2", target_bir_lowering=False, debug=False,
                   num_devices=ndev)
    d_in = {}
    for name, arr in in_maps[0].items():
        dt = {np.dtype(NP8): F8, np.dtype(NPBF16): BF16,
              np.dtype(np.float32): F32}[arr.dtype]
        d_in[name] = nc.dram_tensor(name, list(arr.shape), dt,
                                    kind="ExternalInput")
    d_out = nc.dram_tensor("out", [HID, omax], F32, kind="ExternalOutput")

    with tile.TileContext(nc) as tc:
        pid = nc.partition_id()
        with (
            tc.tile_pool(name="const", bufs=1) as cpool,
            tc.tile_pool(name="sb", bufs=1) as sbp,
            tc.tile_pool(name="psZ", bufs=1, space="PSUM") as psZ,
            tc.tile_pool(name="psW", bufs=1, space="PSUM") as psW,
        ):
            esp_t = cpool.tile([128, 2, cmax], F8, tag="esp")
            wblk_t = cpool.tile([128, smax, 2, 128], F8, tag="wblk")
            w4dr_t = cpool.tile([128, 2, 2, 128], F8, tag="w4dr")
            wcst_t = cpool.tile([128, 256], BF16, tag="wcst")
            fcst_t = cpool.tile([128, 2 + nslmax], F32, tag="fcst")
            nc.sync.dma_start(w4dr_t[:], d_in["w4dr"].ap())
            nc.sync.dma_start(wcst_t[:], d_in["wcst"].ap())
            nc.sync.dma_start(fcst_t[:], d_in["fcst"].ap())
            b5c = fcst_t[:, 0:1]
            b6c = fcst_t[:, 1:2]
            w5b = wcst_t[:, 0:128]
            w6b = wcst_t[:, 128:256]

            def build_core(c):
                # stream inputs per slice on alternating queues
                qeng = [nc.sync, nc.scalar, nc.gpsimd]
                for si, it in enumerate(sched[c]):
                    g, ne, L = it["g"], it["ne"], it["L"]
                    e_lo = it["eoff"]
                    qeng[si % 3].dma_start(
                        esp_t[:, :, e_lo:e_lo + L * it["nep"]],
                        d_in["esp"].ap()[:, :, e_lo:e_lo + L * it["nep"]])
                    s_lo = it["sbase"]
                    s_n = sum(
                        -(-min(Ba[g] - 30 * l, 30) // 3) for l in range(L))
                    qeng[(si + 1) % 3].dma_start(
                        wblk_t[:, s_lo:s_lo + s_n, :, :],
                        d_in["wblk"].ap()[:, s_lo:s_lo + s_n, :, :])

                # ---- flat software-pipelined stream over all chunks ----
                # Per chunk: pairs of betas -> 2 DR z-matmuls into a psum
                # pair tile; one relu per pair (ACT/DVE alternating); W4
                # hi+lo DR matmuls deferred by DEPTH pairs so the in-order
                # PE never waits on a fresh relu.  Out-stage ops of chunk k
                # are emitted one per pair-slot during chunk k+1.
                DEPTH = 2
                chunks = []
                for si, it in enumerate(sched[c]):
                    g, ne = it["g"], it["ne"]
                    ba = Ba[g]
                    lim = os.environ.get("KERNEL_LIMIT_BA")
                    if lim is not None:
                        ba = min(ba, int(lim))
                    for c0 in range(0, ne, ECHUNK):
                        n = min(ECHUNK, ne - c0)
                        chunks.append((it, si, c0, n, ba))

                credit = 0.0
                pending = []      # deferred closures (out-stage ops)
                inflight = []     # (rp, n, first, last) awaiting W4

                def emit_pending():
                    if pending:
                        pending.pop(0)()

                def emit_w4(pW4):
                    rp, n, first, last = inflight.pop(0)
                    nc.tensor.matmul(
                        pW4[:, :n], w4dr_t[:, 0, :, :], rp[:, :, :n],
                        start=first, stop=False, perf_mode=PM.DoubleRow)
                    nc.tensor.matmul(
                        pW4[:, :n], w4dr_t[:, 1, :, :], rp[:, :, :n],
                        start=False, stop=last, perf_mode=PM.DoubleRow)

                for it, si, c0, n, ba in chunks:
                    vb_col = fcst_t[:, 2 + si:3 + si]
                    npair = (ba + 1) // 2
                    # small chunks can't absorb the deferred out-stage of
                    # the previous chunk; drain to keep pool rotation sane
                    while len(pending) > max(0, npair - 1):
                        emit_pending()
                    pW4 = psW.tile([128, ECHUNK], F32, tag="pW4", bufs=2)
                    for pi in range(npair):
                        b0 = 2 * pi
                        cnt = min(2, ba - b0)
                        pz = psZ.tile([128, 2, ECHUNK], F32, tag="pz",
                                      bufs=3)
                        for kk in range(cnt):
                            b = b0 + kk
                            l, rr = b // 30, b % 30
                            q, j = rr % 3, rr // 3
                            sc = it["sbase"] + it["lslots"][l] + j
                            ec = it["eoff"] + l * it["nep"] + c0
                            nc.tensor.matmul(
                                pz[:, kk, :n],
                                wblk_t[32 * q:32 * q + 32, sc, :, :],
                                esp_t[32 * q:32 * q + 32, :, ec:ec + n],
                                start=True, stop=True,
                                perf_mode=PM.DoubleRow,
                                tile_position=(32 * q, 0))
                        rp = sbp.tile([128, 2, ECHUNK], F8, tag="rp",
                                      bufs=6)
                        if cnt == 1:
                            nc.gpsimd.memset(rp[:, 1, :n], 0.0)
                        credit += ALPHA
                        if credit >= 1.0:
                            credit -= 1.0
                            nc.scalar.activation(
                                rp[:, :cnt, :n], pz[:, :cnt, :n],
                                AF.Relu, bias=0.0, scale=1.0)
                        else:
                            nc.vector.tensor_scalar(
                                rp[:, :cnt, :n], pz[:, :cnt, :n],
                                0.0, None, ALU.max)
                        inflight.append(
                            (rp, n, pi == 0, pi == npair - 1))
                        if len(inflight) > DEPTH:
                            emit_w4(pW4)
                        emit_pending()
                    while inflight:
                        emit_w4(pW4)

                    # ---- defer this chunk's out-stage ----
                    def make_out(it=it, c0=c0, n=n, pW4=pW4, vb_col=vb_col):
                        st = {}

                        def s1():
                            st["r5"] = sbp.tile([128, ECHUNK], BF16,
                                                tag="r5", bufs=2, name="r5")
                            nc.vector.tensor_scalar(
                                st["r5"][:, :n], pW4[:, :n], vb_col, None,
                                ALU.add)

                        def s2():
                            st["po"] = psZ.tile([128, 2, ECHUNK], F32,
                                                tag="pz", bufs=3, name="po")
                            nc.tensor.matmul(
                                st["po"][:, 0, :n], w5b, st["r5"][:, :n],
                                start=True, stop=True)

                        def s3():
                            st["r6"] = sbp.tile([128, ECHUNK], BF16,
                                                tag="r6", bufs=2, name="r6")
                            nc.scalar.activation(
                                st["r6"][:, :n], st["po"][:, 0, :n],
                                AF.Relu, bias=b5c, scale=1.0)

                        def s4():
                            nc.tensor.matmul(
                                st["po"][:, 1, :n], w6b, st["r6"][:, :n],
                                start=True, stop=True)

                        def s5():
                            o_sb = sbp.tile([128, ECHUNK], F32, tag="o",
                                            bufs=2, name="o_sb")
                            nc.scalar.activation(
                                o_sb[:, :n], st["po"][:, 1, :n],
                                AF.Identity, bias=b6c, scale=1.0)
                            oc = it["ooff"] + c0
                            nc.sync.dma_start(d_out.ap()[:, oc:oc + n],
                                              o_sb[:, :n])

                        return [s1, s2, s3, s4, s5]

                    pending.extend(make_out())
                while pending:
                    pending.pop(0)()

            if one_core is not None:
                build_core(int(one_core))
            else:
                for case in tc.Switch(pid, NCORES):
                    build_core(case)

    global LAST_NC, LAST_INMAPS, LAST_SCHED
    LAST_NC, LAST_INMAPS, LAST_SCHED = nc, in_maps, sched
    if os.environ.get("KERNEL_BUILD_ONLY"):
        return np.zeros((B * MAX_E, HID), np.float32)
    nc.compile()
    if os.environ.get("KERNEL_COMPILE_ONLY"):
        import tempfile
        neff = bass_utils.compile_bass_kernel(nc, tempfile.mkdtemp())
        print("NEFF:", neff)
        return np.zeros((B * MAX_E, HID), np.float32)
    trace = bool(os.environ.get("KERNEL_TRACE"))
    run_maps = [in_maps[int(one_core)]] if one_core is not None else in_maps
    res = bass_utils.run_bass_kernel_spmd(
        nc, run_maps, core_ids=list(range(len(run_maps))),
        trace=trace,
        trace_cores=list(range(len(run_maps))) if trace else None,
    )
    global LAST_EXEC_NS, LAST_RESULTS
    LAST_RESULTS = res
    LAST_EXEC_NS = res.exec_time_ns

    out = np.zeros((B * MAX_E, HID), np.float32)
    core_list = [int(one_core)] if one_core is not None else range(NCORES)
    for ci, c in enumerate(core_list):
        oc = res.results[ci]["out"]
        for it in sched[c]:
            g, e0, e1 = it["g"], it["e0"], it["e1"]
            out[g * MAX_E + e0:g * MAX_E + e1] = \
                oc[:, it["ooff"]:it["ooff"] + (e1 - e0)].T
    return out


# revision 25
# speedup vs baseline: 1.0615x; 1.0615x over previous
"""Trainium2 Bass kernel for nn_CycleNet_EPD (ragged graph edge-phase decoder).

Math (per graph b, La = edge_len[b], Ba = beta_len[b]):
  ef[e,:4]   = [x[src_e], x[dst_e]]
  s[beta,:]  = sum_e |SCB[b,beta,e]| * ef[e,:]
  emb        = relu(s@W1+b1)@W2+b2 ;  A = emb@W3[:64] + b3
  z_b[:,e]   = A[:,b] + |SCB[b,b,e]| * (W3[64:]^T ef[e])
  H[e,:]     = sum_b relu(z_b)
  out[e,:]   = relu((H@W4 + vb)@W5+b5)@W6+b6,  rows e >= La zero
               vb = 64*b4 + (64-Ba)*relu(A_pad)@W4

v4 design (measured-HW-calibrated):
  - z_b via one K=128 fp8 DoubleRow matmul per beta with CONSTANT masked
    weights (slot s=b%32 occupies rows 2s..2s+1; the 4 edge features ride
    the two k-tiles).  esp[2s+k, kt, e] = fp8(|scb|*ef[2k+kt]) is host-
    packed; betas >=32 use a second esp column layer.  K=128 avoids the
    2x-slow 32-row tile path; DR halves the rhs bytes.
  - relu+bias split over two streams chosen per beta (fixed per slice):
      ACT:  r_b = Relu(pz + A_b)    -> fp8 pair slots; deferred (W4hi,W4hi)
            + (W4lo,W4lo) DoubleRow matmuls fold pairs into PSUM (hi/lo
            keeps W4 at ~bf16 accuracy).
      DVE:  acc = max(pz, -A_b) + acc   (one fused scalar_tensor_tensor,
            using relu(z+A) = max(z,-A) + A; the sum of A's is folded into
            vb on the host).  acc joins via one fp32r matmul per chunk.
  - software pipeline: the in-order PE never waits on a fresh relu (W4
    folds deferred by DEPTH pairs; out-stage ops of chunk k emitted one
    per pair-slot during chunk k+1).
  - out stage: r5 = pW4+vb (DVE), W5 (bf16), r6 = relu(+b5) (ACT), W6
    (bf16), out = +b6 (ACT, fp32); host transposes rows on unshard.

Sharding: per-core (graph, e0, e1) slices balanced by (Ba+OUT_W)*ne+GFIX;
one NEFF with a partition-id switch.  Host does gathers/abs/products/
packing/casts plus the tiny O(Ba*128) prolog (s, emb, A, vb); all
O(Ba*La*128) math runs on the device.
"""

import os
import sys

sys.path.insert(0, "/opt/trn_rl_repo")

import ml_dtypes
import numpy as np

import concourse.bacc as bacc
import concourse.mybir as mybir
import concourse.tile as tile
from concourse import bass_utils

B, MAX_N, MAX_E, MAX_BETA = 16, 512, 1024, 64
NODE_F, HID = 2, 128
NCORES = 8
F32 = mybir.dt.float32
F32R = mybir.dt.float32r
BF16 = mybir.dt.bfloat16
F8 = mybir.dt.float8e4
AF = mybir.ActivationFunctionType
ALU = mybir.AluOpType
PM = mybir.MatmulPerfMode
NPBF16 = ml_dtypes.bfloat16
NP8 = ml_dtypes.float8_e4m3

ECHUNK = 512
OUT_W = 8      # planner: out-stage cost per edge column (beta-col units)
GFIX = 2000    # planner: per-slice fixed cost
ALPHA = 0.46   # fraction of betas on the ACT stream (rest on DVE)
DEPTH = 2      # pairs of deferred W4 folding


def _q8(a):
    return np.clip(a, -240, 240).astype(NP8)


def _assign(ba):
    """Deterministic per-slice engine walk: True = ACT stream."""
    credit, out = 0.0, []
    for _ in range(ba):
        credit += ALPHA
        if credit >= 1.0:
            credit -= 1.0
            out.append(True)
        else:
            out.append(False)
    return out


def _plan(edge_len, beta_len):
    La = [max(1, min(MAX_E, int(v))) for v in edge_len]
    Ba = [max(1, min(MAX_BETA, int(v))) for v in beta_len]

    def el(g, ne):
        return (Ba[g] + OUT_W) * ne + GFIX

    total = sum(el(g, La[g]) for g in range(B))
    target = total / NCORES
    pieces = []
    for g in range(B):
        k = max(1, min(round(el(g, La[g]) / target + 0.25), -(-La[g] // 64)))
        base, rem = divmod(La[g], k)
        e0 = 0
        for j in range(k):
            ne = base + (1 if j < rem else 0)
            pieces.append((g, e0, e0 + ne))
            e0 += ne
    pieces.sort(key=lambda p: -el(p[0], p[2] - p[1]))
    cores = [[] for _ in range(NCORES)]
    loads = [0.0] * NCORES
    for p in pieces:
        c = min(range(NCORES), key=lambda i: loads[i])
        cores[c].append(p)
        loads[c] += el(p[0], p[2] - p[1])
    for _ in range(64):
        cM = max(range(NCORES), key=lambda i: loads[i])
        cm = min(range(NCORES), key=lambda i: loads[i])
        surplus = loads[cM] - loads[cm]
        best = None
        for idx, (g, e0, e1) in enumerate(cores[cM]):
            ne_mv = int((surplus / 2 - GFIX) / (Ba[g] + OUT_W))
            ne_mv = min(ne_mv, e1 - e0 - 64)
            if ne_mv >= 64 and (best is None or ne_mv > best[1]):
                best = (idx, ne_mv)
        if best is None:
            break
        idx, ne_mv = best
        g, e0, e1 = cores[cM][idx]
        cores[cM][idx] = (g, e0, e1 - ne_mv)
        cores[cm].append((g, e1 - ne_mv, e1))
        loads[cM] -= (Ba[g] + OUT_W) * ne_mv
        loads[cm] += el(g, ne_mv)
    return La, Ba, cores


def kernel(x, SCB, edge_index, edge_len, beta_len,
           W1, b1, W2, b2, W3, b3, W4, b4, W5, b5, W6, b6):
    x = np.asarray(x, np.float32)
    SCB = np.asarray(SCB, np.float32)
    edge_index = np.asarray(edge_index, np.int32)
    W1, b1 = np.asarray(W1, np.float32), np.asarray(b1, np.float32)
    W2, b2 = np.asarray(W2, np.float32), np.asarray(b2, np.float32)
    W3, b3 = np.asarray(W3, np.float32), np.asarray(b3, np.float32)
    W4, b4 = np.asarray(W4, np.float32), np.asarray(b4, np.float32)
    W5, b5 = np.asarray(W5, np.float32), np.asarray(b5, np.float32)
    W6, b6 = np.asarray(W6, np.float32), np.asarray(b6, np.float32)
    La, Ba, cores = _plan(np.asarray(edge_len), np.asarray(beta_len))
    W3a, W3b = W3[:64], W3[64:]
    W3b8 = _q8(W3b).astype(np.float32)

    # ---- host prolog per graph ----
    ef_all, A_all, vb_all = [], [], []
    K0 = np.maximum((np.maximum(b1, 0) @ W2 + b2) @ W3a + b3, 0.0)
    for g in range(B):
        la, ba = La[g], Ba[g]
        src, dst = edge_index[g, 0, :la], edge_index[g, 1, :la]
        ef = np.concatenate([x[g][src], x[g][dst]], axis=1)
        ef_all.append(ef)
        asc = np.abs(SCB[g][:ba, :la])
        s = asc @ ef
        emb = np.maximum(s @ W1 + b1, 0.0) @ W2 + b2
        A_all.append(emb @ W3a + b3)                       # [ba,128]
        vb_all.append((64.0 - ba) * (K0 @ W4) + 64.0 * b4)

    # ---- per-core layouts ----
    sched = []
    cmax = omax = amax = nslmax = 1
    for c in range(NCORES):
        eoff = ooff = aoff = 0
        items = []
        for (g, e0, e1) in cores[c]:
            ne = e1 - e0
            nep = -(-ne // 8) * 8
            ba = Ba[g]
            L = -(-ba // 32)
            items.append(dict(g=g, e0=e0, e1=e1, ne=ne, nep=nep, L=L,
                              eoff=eoff, ooff=ooff, aoff=aoff))
            eoff += L * nep
            ooff += nep
            aoff += ba
        sched.append(items)
        cmax = max(cmax, -(-eoff // 8) * 8)
        omax = max(omax, ooff)
        amax = max(amax, aoff)
        nslmax = max(nslmax, len(items))

    # constant masked z-weights: slot s rows 2s..2s+1; ktile pairs carry
    # the 4 edge features
    w3bm = np.zeros((128, 32, 2, 128), NP8)
    for s in range(32):
        w3bm[2 * s + 0, s, 0] = W3b8[0]
        w3bm[2 * s + 0, s, 1] = W3b8[1]
        w3bm[2 * s + 1, s, 0] = W3b8[2]
        w3bm[2 * s + 1, s, 1] = W3b8[3]
    w4hi = _q8(W4)
    w4lo = _q8(W4 - w4hi.astype(np.float32))
    w4dr = np.zeros((128, 2, 2, 128), NP8)
    w4dr[:, 0, 0] = w4hi
    w4dr[:, 0, 1] = w4hi
    w4dr[:, 1, 0] = w4lo
    w4dr[:, 1, 1] = w4lo
    wcst = np.zeros((128, 384), np.float32)
    wcst[:, 0:128] = W5
    wcst[:, 128:256] = W6
    wcst[:, 256:384] = W4

    in_maps = []
    for c in range(NCORES):
        esp = np.zeros((128, 2, cmax), NP8)
        acst = np.zeros((128, amax), np.float32)
        fcst = np.zeros((128, 2 + nslmax), np.float32)
        fcst[:, 0] = b5
        fcst[:, 1] = b6
        for si, it in enumerate(sched[c]):
            g, e0, e1, ne = it["g"], it["e0"], it["e1"], it["ne"]
            ba = Ba[g]
            A = A_all[g]
            asn = _assign(ba)
            # vb'' = vb + (sum of DVE-stream A_b) @ W4
            Adve = sum(A[b] for b in range(ba) if not asn[b])
            if not isinstance(Adve, np.ndarray):
                Adve = np.zeros(128, np.float32)
            fcst[:, 2 + si] = vb_all[g] + Adve @ W4
            for b in range(ba):
                acst[:, it["aoff"] + b] = A[b] if asn[b] else -A[b]
            ef = ef_all[g][e0:e1]
            asc = np.abs(SCB[g][:ba, e0:e1])
            for b in range(ba):
                s_, l = b % 32, b // 32
                col = it["eoff"] + l * it["nep"]
                prod = _q8(asc[b][None, :] * ef.T)      # [4,ne]
                esp[2 * s_ + 0, 0, col:col + ne] = prod[0]
                esp[2 * s_ + 0, 1, col:col + ne] = prod[1]
                esp[2 * s_ + 1, 0, col:col + ne] = prod[2]
                esp[2 * s_ + 1, 1, col:col + ne] = prod[3]
        in_maps.append({
            "esp": esp, "w3bm": w3bm, "w4dr": w4dr,
            "wcst": wcst.astype(np.float32), "acst": acst, "fcst": fcst,
        })

    # ---- build program ----
    one_core = os.environ.get("KERNEL_ONE_CORE")
    ndev = 1 if one_core is not None else NCORES
    nc = bacc.Bacc("TRN2", target_bir_lowering=False, debug=False,
                   num_devices=ndev)
    d_in = {}
    for name, arr in in_maps[0].items():
        dt = {np.dtype(NP8): F8, np.dtype(NPBF16): BF16,
              np.dtype(np.float32): F32}[arr.dtype]
        d_in[name] = nc.dram_tensor(name, list(arr.shape), dt,
                                    kind="ExternalInput")
    d_out = nc.dram_tensor("out", [HID, omax], F32, kind="ExternalOutput")

    with tile.TileContext(nc) as tc:
        pid = nc.partition_id()
        with (
            tc.tile_pool(name="const", bufs=1) as cpool,
            tc.tile_pool(name="sb", bufs=1) as sbp,
            tc.tile_pool(name="psZ", bufs=1, space="PSUM") as psZ,
            tc.tile_pool(name="psW", bufs=1, space="PSUM") as psW,
        ):
            esp_t = cpool.tile([128, 2, cmax], F8, tag="esp")
            w3bm_t = cpool.tile([128, 32, 2, 128], F8, tag="w3bm")
            w4dr_t = cpool.tile([128, 2, 2, 128], F8, tag="w4dr")
            wcst_t = cpool.tile([128, 384], F32, tag="wcst")
            wcb_t = cpool.tile([128, 384], BF16, tag="wcb")
            acst_t = cpool.tile([128, amax], F32, tag="acst")
            fcst_t = cpool.tile([128, 2 + nslmax], F32, tag="fcst")
            zcol_t = cpool.tile([128, ECHUNK], F32, tag="zcol")
            nc.scalar.dma_start(w3bm_t[:], d_in["w3bm"].ap())
            nc.sync.dma_start(w4dr_t[:], d_in["w4dr"].ap())
            nc.sync.dma_start(wcst_t[:], d_in["wcst"].ap())
            nc.sync.dma_start(acst_t[:], d_in["acst"].ap())
            nc.sync.dma_start(fcst_t[:], d_in["fcst"].ap())
            nc.gpsimd.memset(zcol_t[:], 0.0)
            nc.vector.tensor_copy(wcb_t[:], wcst_t[:])  # bf16 W5|W6|W4
            b5c = fcst_t[:, 0:1]
            b6c = fcst_t[:, 1:2]
            w5b = wcb_t[:, 0:128]
            w6b = wcb_t[:, 128:256]
            w4b = wcb_t[:, 256:384]

            def build_core(c):
                qeng = [nc.sync, nc.scalar, nc.gpsimd]
                for si, it in enumerate(sched[c]):
                    e_lo = it["eoff"]
                    ln = it["L"] * it["nep"]
                    qeng[si % 3].dma_start(
                        esp_t[:, :, e_lo:e_lo + ln],
                        d_in["esp"].ap()[:, :, e_lo:e_lo + ln])

                chunks = []
                for si, it in enumerate(sched[c]):
                    ba = Ba[it["g"]]
                    lim = os.environ.get("KERNEL_LIMIT_BA")
                    if lim is not None:
                        ba = min(ba, int(lim))
                    for c0 in range(0, it["ne"], ECHUNK):
                        n = min(ECHUNK, it["ne"] - c0)
                        chunks.append((it, si, c0, n, ba))

                pending = []

                def emit_pending():
                    if pending:
                        pending.pop(0)()

                for it, si, c0, n, ba in chunks:
                    vb_col = fcst_t[:, 2 + si:3 + si]
                    asn = _assign(ba)
                    nact = sum(asn)
                    napair = (nact + 1) // 2
                    while len(pending) > max(0, ba - 1):
                        emit_pending()
                    pW4 = psW.tile([128, ECHUNK], F32, tag="pW4", bufs=2)
                    started = [False]

                    inflight = []   # completed ACT rp pairs awaiting W4

                    def emit_w4(last):
                        rp = inflight.pop(0)
                        nc.tensor.matmul(
                            pW4[:, :n], w4dr_t[:, 0, :, :], rp[:, :, :n],
                            start=not started[0], stop=False,
                            perf_mode=PM.DoubleRow)
                        started[0] = True
                        nc.tensor.matmul(
                            pW4[:, :n], w4dr_t[:, 1, :, :], rp[:, :, :n],
                            start=False, stop=last,
                            perf_mode=PM.DoubleRow)

                    acc = None
                    act_i = 0
                    rp_cur = None
                    ndve = ba - nact
                    w4_emitted = 0
                    for b in range(ba):
                        s_, l = b % 32, b // 32
                        ec = it["eoff"] + l * it["nep"] + c0
                        pz = psZ.tile([128, ECHUNK], F32, tag="pz", bufs=5)
                        nc.tensor.matmul(
                            pz[:, :n], w3bm_t[:, s_, :, :],
                            esp_t[:, :, ec:ec + n],
                            start=True, stop=True, perf_mode=PM.DoubleRow)
                        a_col = acst_t[:, it["aoff"] + b:
                                       it["aoff"] + b + 1]
                        if asn[b]:
                            kk = act_i % 2
                            if kk == 0:
                                rp_cur = sbp.tile([128, 2, ECHUNK], F8,
                                                  tag="rp", bufs=6,
                                                  name="rp")
                            nc.scalar.activation(
                                rp_cur[:, kk, :n], pz[:, :n],
                                AF.Relu, bias=a_col, scale=1.0)
                            act_i += 1
                            if kk == 1:
                                inflight.append(rp_cur)
                            elif act_i == nact:   # odd straggler
                                nc.gpsimd.memset(rp_cur[:, 1, :n], 0.0)
                                inflight.append(rp_cur)
                            if len(inflight) > DEPTH:
                                last = (w4_emitted == napair - 1
                                        and ndve == 0)
                                emit_w4(last)
                                w4_emitted += 1
                        else:
                            first = acc is None
                            if first:
                                acc = sbp.tile([128, ECHUNK], F32,
                                               tag="acc", bufs=2,
                                               name="acc")
                            nc.vector.scalar_tensor_tensor(
                                acc[:, :n], pz[:, :n], a_col,
                                zcol_t[:, :n] if first else acc[:, :n],
                                ALU.max, ALU.add)
                        emit_pending()
                    while inflight:
                        last = (w4_emitted == napair - 1 and ndve == 0)
                        emit_w4(last)
                        w4_emitted += 1
                    if ndve > 0:
                        # join the DVE accumulator: pW4 += bf16(acc) @ W4
                        accb = sbp.tile([128, ECHUNK], BF16, tag="accb",
                                        bufs=2, name="accb")
                        nc.scalar.activation(accb[:, :n], acc[:, :n],
                                             AF.Identity, bias=0.0,
                                             scale=1.0)
                        nc.tensor.matmul(
                            pW4[:, :n], w4b, accb[:, :n],
                            start=not started[0], stop=True)
                        started[0] = True

                    # ---- deferred out stage ----
                    def make_out(it=it, c0=c0, n=n, pW4=pW4, vb_col=vb_col):
                        st = {}

                        def s1():
                            st["r5"] = sbp.tile([128, ECHUNK], BF16,
                                                tag="r5", bufs=2, name="r5")
                            nc.vector.tensor_scalar(
                                st["r5"][:, :n], pW4[:, :n], vb_col, None,
                                ALU.add)

                        def s2():
                            st["p2"] = psZ.tile([128, ECHUNK], F32,
                                                tag="pz", bufs=5, name="p2")
                            nc.tensor.matmul(
                                st["p2"][:, :n], w5b, st["r5"][:, :n],
                                start=True, stop=True)

                        def s3():
                            st["r6"] = sbp.tile([128, ECHUNK], BF16,
                                                tag="r6", bufs=2, name="r6")
                            nc.scalar.activation(
                                st["r6"][:, :n], st["p2"][:, :n],
                                AF.Relu, bias=b5c, scale=1.0)

                        def s4():
                            st["p3"] = psZ.tile([128, ECHUNK], F32,
                                                tag="pz", bufs=5, name="p3")
                            nc.tensor.matmul(
                                st["p3"][:, :n], w6b, st["r6"][:, :n],
                                start=True, stop=True)

                        def s5():
                            o_sb = sbp.tile([128, ECHUNK], F32, tag="o",
                                            bufs=2, name="o_sb")
                            nc.scalar.activation(
                                o_sb[:, :n], st["p3"][:, :n],
                                AF.Identity, bias=b6c, scale=1.0)
                            oc = it["ooff"] + c0
                            nc.sync.dma_start(d_out.ap()[:, oc:oc + n],
                                              o_sb[:, :n])

                        return [s1, s2, s3, s4, s5]

                    pending.extend(make_out())
                while pending:
                    pending.pop(0)()

            if one_core is not None:
                build_core(int(one_core))
            else:
                for case in tc.Switch(pid, NCORES):
                    build_core(case)

    global LAST_NC, LAST_INMAPS, LAST_SCHED
    LAST_NC, LAST_INMAPS, LAST_SCHED = nc, in_maps, sched
    if os.environ.get("KERNEL_BUILD_ONLY"):
        return np.zeros((B * MAX_E, HID), np.float32)
    nc.compile()
    trace = bool(os.environ.get("KERNEL_TRACE"))
    run_maps = [in_maps[int(one_core)]] if one_core is not None else in_maps
    res = bass_utils.run_bass_kernel_spmd(
        nc, run_maps, core_ids=list(range(len(run_maps))),
        trace=trace,
        trace_cores=list(range(len(run_maps))) if trace else None,
    )
    global LAST_EXEC_NS, LAST_RESULTS
    LAST_RESULTS = res
    LAST_EXEC_NS = res.exec_time_ns

    out = np.zeros((B * MAX_E, HID), np.float32)
    core_list = [int(one_core)] if one_core is not None else range(NCORES)
    for ci, c in enumerate(core_list):
        oc = res.results[ci]["out"]
        for it in sched[c]:
            g, e0, e1 = it["g"], it["e0"], it["e1"]
            out[g * MAX_E + e0:g * MAX_E + e1] = \
                oc[:, it["ooff"]:it["ooff"] + (e1 - e0)].T
    return out
